# revision 4
# baseline (speedup 1.0000x reference)
"""Self-contained Trainium2 Bass kernel for nn_AttnLayer_71382356460296.

Sharding: data-parallel over batch B (2) x sequence-parallel over query
chunks (4) => 8 cores. Each core computes its (batch, 1024-query chunk)
slice of the full layer: q-projection + RoPE, windowed GQA attention with
a top-left-aligned causal mask, and the output projection. No reduction
is needed across cores - the host just concatenates the 8 output chunks.

All matmuls run in float32r (full-rate fp32 on the PE, ~1.6e-4 rel err).
Scores are computed transposed (keys on partitions, queries free) so the
softmax denominator comes from a ones-column in V and the exp runs on the
scalar engine straight out of PSUM. The causal mask is applied with a
bias-row matmul accumulation (column mask) plus a small triangular
multiply on the diagonal 128-blocks.
"""

import numpy as np

import concourse.bacc as bacc
import concourse.mybir as mybir
import concourse.tile as tile
from concourse.bass_utils import run_bass_kernel_spmd

F32 = mybir.dt.float32
F32R = mybir.dt.float32r
AF = mybir.ActivationFunctionType
OP = mybir.AluOpType

FULL = dict(B=2, T=4096, D=2048, H=32, KV=8, DH=64, W=1024, BASE=10000.0)
BIGNEG = -1e30


def _derived(cfg):
    d = dict(cfg)
    d["CH"] = cfg["T"] // 4            # queries per core
    d["KB"] = cfg["W"] // 128          # 128-key blocks in window
    d["DT"] = cfg["D"] // 128          # contraction tiles for Wq
    d["NP"] = cfg["H"] // 2            # head pairs (= D/128 output tiles)
    d["NC"] = [(i, min(512, d["CH"] - i)) for i in range(0, d["CH"], 512)]
    assert d["NP"] * 128 == cfg["D"] and d["NP"] % 4 == 0
    return d


def build(cfg):
    c = _derived(cfg)
    CH, KB, DT, NP, KV, H = c["CH"], c["KB"], c["DT"], c["NP"], c["KV"], c["H"]
    hpkv = H // KV
    nc = bacc.Bacc("TRN2", target_bir_lowering=False, debug=False)

    xT = nc.dram_tensor("xT", [c["D"], CH], F32R, kind="ExternalInput")
    wqT = nc.dram_tensor("wqT", [c["D"], c["D"]], F32R, kind="ExternalInput")
    woT = nc.dram_tensor("woT", [c["D"], c["D"]], F32R, kind="ExternalInput")
    kT = nc.dram_tensor("kT", [64, KV, KB, 128], F32R, kind="ExternalInput")
    kTd = nc.dram_tensor("kTd", [64, KV, KB, 128], F32R, kind="ExternalInput")
    vaug = nc.dram_tensor("vaug", [128, KV, KB, 65], F32R, kind="ExternalInput")
    cosT = nc.dram_tensor("cosT", [128, CH], F32, kind="ExternalInput")
    sinT = nc.dram_tensor("sinT", [128, CH], F32, kind="ExternalInput")
    tri = nc.dram_tensor("tri", [128, 128], F32, kind="ExternalInput")
    brow = nc.dram_tensor("brow", [KB, CH], F32R, kind="ExternalInput")
    sel = nc.dram_tensor("sel", [KB, KB, 128], F32R, kind="ExternalInput")
    e8 = nc.dram_tensor("e8", [8, 4, 128], F32R, kind="ExternalInput")
    out = nc.dram_tensor("out", [CH, c["D"]], F32, kind="ExternalOutput")

    swap = [i ^ 1 for i in range(32)]

    with nc.allow_low_precision(reason="fp32r matmuls are intended"), \
         tile.TileContext(nc) as tc:
        with (
            tc.tile_pool(name="consts", bufs=1) as cp,
            tc.tile_pool(name="qt", bufs=1) as qtp,
            tc.tile_pool(name="psbig", bufs=2, space="PSUM") as psb,
            tc.tile_pool(name="psav", bufs=2, space="PSUM") as psa,
            tc.tile_pool(name="dram", bufs=1, space="DRAM") as dp,
        ):
            cos_sb = cp.tile([128, CH], F32)
            nc.sync.dma_start(cos_sb[:], cosT[:])
            sin_sb = cp.tile([128, CH], F32)
            nc.sync.dma_start(sin_sb[:], sinT[:])
            qts = []
            attnT_d = dp.tile([128, NP, CH], F32R)

            # ---- Phase A: q = rope(x @ WqT) in transposed per-pair layout
            with (
                tc.tile_pool(name="xts", bufs=1) as xp,
                tc.tile_pool(name="wq", bufs=3) as wp,
                tc.tile_pool(name="rope", bufs=2) as rp,
            ):
                xts = xp.tile([128, DT, CH], F32R)
                nc.sync.dma_start(
                    xts[:], xT.rearrange("(kt p) f -> p kt f", p=128))
                for m in range(NP):
                    wq_m = wp.tile([128, DT, 128], F32R, tag="wq")
                    nc.sync.dma_start(
                        wq_m[:],
                        wqT.rearrange("(kt p) e -> p kt e", p=128)[
                            :, :, 128 * m:128 * (m + 1)])
                    qp = psb.tile([128, CH], F32, tag="big")
                    for n0, nn in c["NC"]:
                        for kt in range(DT):
                            nc.tensor.matmul(
                                qp[:, n0:n0 + nn], wq_m[:, kt, :],
                                xts[:, kt, n0:n0 + nn],
                                start=(kt == 0), stop=(kt == DT - 1))
                    qcp = rp.tile([128, CH], F32, tag="qcp")
                    nc.scalar.copy(qcp[:], qp[:])
                    t1 = rp.tile([128, CH], F32, tag="t1")
                    nc.vector.tensor_mul(t1[:], qp[:], cos_sb[:])
                    qs = rp.tile([128, CH], F32, tag="qs")
                    nc.vector.stream_shuffle(qs[:], qcp[:], swap)
                    t2 = rp.tile([128, CH], F32, tag="t2")
                    nc.vector.tensor_mul(t2[:], qs[:], sin_sb[:])
                    qt = qtp.tile([128, CH], F32R, tag=f"qt{m}")
                    nc.vector.tensor_add(qt[:], t1[:], t2[:])
                    qts.append(qt)

            # ---- Phase B: attention per head, scores transposed
            with (
                tc.tile_pool(name="kv", bufs=1) as kp,
                tc.tile_pool(name="expp", bufs=3) as ep,
                tc.tile_pool(name="attu", bufs=6) as up,
                tc.tile_pool(name="den", bufs=2) as dnp,
                tc.tile_pool(name="att", bufs=3) as ap,
            ):
                ktd_sb = kp.tile([128, KV, KB, 128], F32R)
                nc.sync.dma_start(ktd_sb[0:64], kT[:])
                nc.sync.dma_start(ktd_sb[64:128], kTd[:])
                va_sb = kp.tile([128, KV, KB, 65], F32R)
                nc.sync.dma_start(va_sb[:], vaug[:])
                tri_sb = kp.tile([128, 128], F32)
                nc.sync.dma_start(tri_sb[:], tri[:])
                br_sb = kp.tile([KB, CH], F32R)
                nc.sync.dma_start(br_sb[:], brow[:])
                sel_sb = kp.tile([KB, KB, 128], F32R)
                nc.sync.dma_start(sel_sb[:], sel[:])
                e8_sb = kp.tile([8, 4, 128], F32R)
                nc.sync.dma_start(e8_sb[:], e8[:])

                for g in range(NP // 4):       # groups of 4 pairs
                    colg = dnp.tile([8, CH], F32, tag="col", bufs=1)
                    aus = []
                    for j in range(4):
                        m = 4 * g + j
                        au = up.tile([128, CH], F32, tag="au")
                        for hh in range(2):
                            h = 2 * m + hh
                            kv = h // hpkv
                            av = psa.tile([128, CH], F32, tag="av")
                            for kb in range(KB):
                                sp = psb.tile([128, CH], F32, tag="big")
                                if hh == 0:
                                    lh = ktd_sb[0:64, kv, kb, :]
                                    rh = qts[m][0:64]
                                else:
                                    lh = ktd_sb[64:128, kv, kb, :]
                                    rh = qts[m][64:128]
                                for n0, nn in c["NC"]:
                                    mb = min(128 * kb, n0 + nn)
                                    has_bias = mb > n0
                                    nc.tensor.matmul(
                                        sp[:, n0:n0 + nn], lh,
                                        rh[:, n0:n0 + nn],
                                        start=True, stop=not has_bias)
                                    if has_bias:
                                        nc.tensor.matmul(
                                            sp[:, n0:mb],
                                            sel_sb[:, kb, :],
                                            br_sb[:, n0:mb],
                                            start=False, stop=True)
                                er = ep.tile([128, CH], F32R, tag="er")
                                nc.scalar.activation(er[:], sp[:], AF.Exp)
                                if 128 * (kb + 1) <= CH:
                                    dsl = slice(128 * kb, 128 * (kb + 1))
                                    nc.vector.tensor_mul(
                                        er[:, dsl], er[:, dsl], tri_sb[:])
                                for n0, nn in c["NC"]:
                                    nc.tensor.matmul(
                                        av[0:65, n0:n0 + nn],
                                        va_sb[:, kv, kb, :],
                                        er[:, n0:n0 + nn],
                                        start=(kb == 0), stop=(kb == KB - 1))
                            nc.vector.tensor_copy(
                                au[64 * hh:64 * (hh + 1)], av[0:64])
                            stg = dnp.tile([128, CH], F32, tag="stg")
                            nc.scalar.copy(stg[64:65], av[64:65])
                            nc.sync.dma_start(
                                colg[2 * j + hh:2 * j + hh + 1], stg[64:65])
                        aus.append(au)
                    recg = dnp.tile([8, CH], F32R, tag="rec", bufs=1)
                    nc.vector.reciprocal(recg[:], colg[:])
                    for j in range(4):
                        m = 4 * g + j
                        bc = psa.tile([128, CH], F32, tag="av")
                        for n0, nn in c["NC"]:
                            nc.tensor.matmul(
                                bc[:, n0:n0 + nn], e8_sb[:, j, :],
                                recg[:, n0:n0 + nn], start=True, stop=True)
                        at = ap.tile([128, CH], F32R, tag="at")
                        nc.vector.tensor_mul(at[:], aus[j][:], bc[:])
                        nc.sync.dma_start(attnT_d[:, m, :], at[:])

            # ---- Phase C: out = attnT.T @ WoT
            with (
                tc.tile_pool(name="wo", bufs=1) as wop,
                tc.tile_pool(name="acol", bufs=3) as acp,
                tc.tile_pool(name="osb", bufs=3) as op_,
            ):
                MQ = CH // 128
                OH = c["D"] // 2
                for nh in range(2):
                    wo_h = wop.tile([128, DT, OH], F32R, tag="wo")
                    nc.sync.dma_start(
                        wo_h[:],
                        woT.rearrange("(kt p) o -> p kt o", p=128)[
                            :, :, OH * nh:OH * (nh + 1)])
                    for mq in range(MQ):
                        acol = acp.tile([128, NP, 128], F32R, tag="ac")
                        nc.sync.dma_start(
                            acol[:],
                            attnT_d[:, :, 128 * mq:128 * (mq + 1)])
                        for o0 in range(0, OH, 512):
                            ow = min(512, OH - o0)
                            opx = psb.tile([128, 512], F32, tag="big")
                            for kq in range(NP):
                                nc.tensor.matmul(
                                    opx[:, :ow], acol[:, kq, :],
                                    wo_h[:, kq, o0:o0 + ow],
                                    start=(kq == 0), stop=(kq == NP - 1))
                            osb = op_.tile([128, 512], F32, tag="os")
                            nc.vector.tensor_copy(osb[:, :ow], opx[:, :ow])
                            nc.sync.dma_start(
                                out[128 * mq:128 * (mq + 1),
                                    OH * nh + o0:OH * nh + o0 + ow],
                                osb[:, :ow])
    nc.compile()
    return nc


def host_inputs(cfg, x, k_cache, v_cache, Wq, Wo, core):
    c = _derived(cfg)
    CH, KB, KV, W, DH = c["CH"], c["KB"], c["KV"], c["W"], c["DH"]
    b, ch = core // 4, core % 4
    Tc = k_cache.shape[2]
    f32 = np.float32

    xT = np.ascontiguousarray(x[b, CH * ch:CH * (ch + 1), :].T).astype(f32)
    wqT = np.ascontiguousarray(Wq.T).astype(f32) * f32(1.0 / np.sqrt(DH))
    woT = np.ascontiguousarray(Wo.T).astype(f32)
    kw = k_cache[b, :, Tc - W:, :]                      # (KV, W, DH)
    kT = np.ascontiguousarray(
        kw.reshape(KV, KB, 128, DH).transpose(3, 0, 1, 2)).astype(f32)
    vw = v_cache[b, :, Tc - W:, :].reshape(KV, KB, 128, DH)
    vaug = np.ones((128, KV, KB, 65), f32)
    vaug[:, :, :, :DH] = vw.transpose(2, 0, 1, 3)
    pos = (CH * ch + np.arange(CH)).astype(f32)
    inv = 1.0 / (cfg["BASE"] ** (np.arange(0, DH, 2, dtype=f32) / DH))
    r = np.arange(128)
    u = (r % 64) // 2
    ang = pos[None, :] * inv[u][:, None]                # (128, CH)
    cosT = np.cos(ang).astype(f32)
    sinT = (np.sin(ang) * np.where(r % 2 == 0, -1.0, 1.0)[:, None]).astype(f32)
    if ch == 0:
        tri = (np.arange(128)[:, None] <= np.arange(128)[None, :]).astype(f32)
        brow = np.zeros((KB, CH), f32)
        for kb in range(KB):
            brow[kb, :128 * kb] = BIGNEG
    else:
        tri = np.ones((128, 128), f32)
        brow = np.zeros((KB, CH), f32)
    sel = np.zeros((KB, KB, 128), f32)
    for kb in range(KB):
        sel[kb, kb, :] = 1.0
    e8 = np.zeros((8, 4, 128), f32)
    for j in range(4):
        e8[2 * j, j, :64] = 1.0
        e8[2 * j + 1, j, 64:] = 1.0
    return {"xT": xT, "wqT": wqT, "woT": woT, "kT": kT, "kTd": kT,
            "vaug": vaug, "cosT": cosT, "sinT": sinT, "tri": tri,
            "brow": brow, "sel": sel, "e8": e8}


_NC_CACHE = {}


def run(cfg, x, k_cache, v_cache, Wq, Wo, trace=False):
    key = tuple(sorted((k, v) for k, v in cfg.items()))
    if key not in _NC_CACHE:
        _NC_CACHE[key] = build(cfg)
    nc = _NC_CACHE[key]
    in_maps = [host_inputs(cfg, x, k_cache, v_cache, Wq, Wo, c)
               for c in range(8)]
    res = run_bass_kernel_spmd(nc, in_maps, core_ids=list(range(8)),
                               trace=trace)
    outs = [res.results[c]["out"] for c in range(8)]
    full = np.stack([np.concatenate(outs[0:4], axis=0),
                     np.concatenate(outs[4:8], axis=0)])
    return full, res


def kernel(x, k_cache, v_cache, Wq, Wo):
    full, _ = run(FULL, np.asarray(x), np.asarray(k_cache),
                  np.asarray(v_cache), np.asarray(Wq), np.asarray(Wo))
    return full.astype(np.float32)
